# revision 1
# baseline (speedup 1.0000x reference)
"""Multi-head sparse attention on 8 NeuronCores (Trainium2, Bass/Tile).

Head-parallel sharding: core h owns head h (H == n_cores == 8).
Each core computes its head's attention output and the partial final
projection through its W_O column slice; the host sums the 8 partials.

Math note: softmax rows are never fully masked (random 0/1 mask) and
E = QK^T/8 has |E| <~ 6, so softmax is computed WITHOUT max subtraction:
P = exp(E) * mask, out = (P @ V) / rowsum(P).  rowsum is obtained by
appending a ones-column to V, and the normalization is deferred until
after the W_O projection (a per-row scalar).
"""

import numpy as np
import ml_dtypes

H, N, F_IN, HD, F_OUT = 8, 4096, 512, 64, 512
N_CORES = 8
RG = 1024            # query-row group processed per PSUM accumulator
N_RG = N // RG       # 4
MC = 128             # key/m chunk (partition dim)
N_MC = N // MC       # 32
NSPLIT = 512         # matmul moving-operand free size
BF16 = ml_dtypes.bfloat16

_PROGRAM_CACHE = {}


def _build_program(repeat=1, timing=False, variant="full"):
    """Build + compile the Bass/Tile program (same SPMD program for all cores).

    timing=True builds a benchmark variant: inputs live in internal DRAM
    (initialized on-device), the body runs `repeat` times inside a hardware
    For_i loop, and only a tiny checksum output is external.  Differencing
    the wall time of two repeat counts isolates the per-iteration HW time.
    """
    key = (repeat, timing, variant)
    if key in _PROGRAM_CACHE:
        return _PROGRAM_CACHE[key]

    import concourse.bacc as bacc
    import concourse.tile as tile
    import concourse.mybir as mybir

    f32 = mybir.dt.float32
    f32r = mybir.dt.float32r
    bf16 = mybir.dt.bfloat16

    nc = bacc.Bacc("TRN2", target_bir_lowering=False, debug=False,
                   num_devices=N_CORES)

    if not timing:
        XT = nc.dram_tensor("xt", [F_IN, N], f32r, kind="ExternalInput").ap()
        MT = nc.dram_tensor("mt", [N, N], bf16, kind="ExternalInput").ap()
        WQ = nc.dram_tensor("wq", [128, 4, HD], f32r, kind="ExternalInput").ap()
        WK = nc.dram_tensor("wk", [128, 4, HD], f32r, kind="ExternalInput").ap()
        WV = nc.dram_tensor("wv", [128, 4, HD], f32r, kind="ExternalInput").ap()
        WO = nc.dram_tensor("wo", [HD, F_OUT], f32r, kind="ExternalInput").ap()
        OUT = nc.dram_tensor("out", [N, F_OUT], f32, kind="ExternalOutput").ap()
    else:
        XT = nc.dram_tensor("xt", [F_IN, N], f32r).ap()
        MT = nc.dram_tensor("mt", [N, N], bf16).ap()
        WQ = nc.dram_tensor("wq", [128, 4, HD], f32r).ap()
        WK = nc.dram_tensor("wk", [128, 4, HD], f32r).ap()
        WV = nc.dram_tensor("wv", [128, 4, HD], f32r).ap()
        WO = nc.dram_tensor("wo", [HD, F_OUT], f32r).ap()
        OUT = nc.dram_tensor("out", [N, F_OUT], f32).ap()
        DUMMY = nc.dram_tensor("dumin", [1, 8], f32, kind="ExternalInput").ap()
        CHK = nc.dram_tensor("chk", [128, F_OUT], f32,
                             kind="ExternalOutput").ap()

    SCALE = float(1.0 / np.sqrt(HD))

    with tile.TileContext(nc) as tc:
        with (
            tc.tile_pool(name="consts", bufs=1) as consts,
            tc.tile_pool(name="wpool", bufs=1) as wpool,
        ):
            ident11 = consts.tile([1, 1], f32)
            nc.vector.memset(ident11[:], 1.0)
            zeros_p = consts.tile([128, 1], f32)
            nc.vector.memset(zeros_p[:], 0.0)

            wq_sb = wpool.tile([128, 4, HD], f32r)
            wk_sb = wpool.tile([128, 4, HD], f32r)
            wv_sb = wpool.tile([128, 4, HD], f32r)
            wo_sb = wpool.tile([HD, F_OUT], f32r)

            if timing:
                # on-device init of internal DRAM inputs (runs once)
                with tc.tile_pool(name="init", bufs=1) as initp:
                    mrow = initp.tile([128, N], bf16)
                    nc.vector.memset(mrow[:], 1.0)
                    for c in range(N_MC):
                        nc.sync.dma_start(MT[c * 128:(c + 1) * 128, :], mrow[:])
                    xrow = initp.tile([128, N], f32r)
                    nc.vector.memset(xrow.bitcast(f32)[:], 0.015625)
                    for c in range(4):
                        nc.sync.dma_start(XT[c * 128:(c + 1) * 128, :], xrow[:])
                    wrow = initp.tile([128, 4 * HD], f32r)
                    nc.vector.memset(wrow.bitcast(f32)[:], 0.03125)
                    for W in (WQ, WK, WV):
                        nc.sync.dma_start(
                            W.rearrange("p c d -> p (c d)"), wrow[:])
                    worow = initp.tile([HD, F_OUT], f32r)
                    nc.vector.memset(worow.bitcast(f32)[:], 0.03125)
                    nc.sync.dma_start(WO[:], worow[:])

            nc.sync.dma_start(wq_sb[:], WQ[:])
            nc.sync.dma_start(wk_sb[:], WK[:])
            nc.sync.dma_start(wv_sb[:], WV[:])
            nc.sync.dma_start(wo_sb[:], WO[:])

            if timing and repeat > 1:
                with tc.For_i(0, repeat, 1):
                    _one_pass(nc, tc, mybir, XT, MT, OUT,
                              wq_sb, wk_sb, wv_sb, wo_sb, ident11, zeros_p,
                              SCALE, 0, variant)
            else:
                for rep in range(repeat):
                    _one_pass(nc, tc, mybir, XT, MT, OUT,
                              wq_sb, wk_sb, wv_sb, wo_sb, ident11, zeros_p,
                              SCALE, rep, variant)

            if timing:
                with tc.tile_pool(name="chkp", bufs=1) as chkp:
                    chk_sb = chkp.tile([128, F_OUT], f32)
                    nc.sync.dma_start(chk_sb[:], OUT[0:128, :])
                    nc.sync.dma_start(CHK[:], chk_sb[:])

    nc.compile()
    _PROGRAM_CACHE[key] = nc
    return nc


def _one_pass(nc, tc, mybir, XT, MT, OUT,
              wq_sb, wk_sb, wv_sb, wo_sb, ident11, zeros_p, SCALE, rep,
              variant="full"):
    f32 = mybir.dt.float32
    f32r = mybir.dt.float32r
    bf16 = mybir.dt.bfloat16
    AF = mybir.ActivationFunctionType
    r = f"_r{rep}"

    if variant == "dmaonly":
        with tc.tile_pool(name="mdma" + r, bufs=4) as mpool:
            for g in range(N_RG):
                r0 = g * RG
                for c in range(N_MC):
                    mt_t = mpool.tile([128, RG], bf16, name="mt_t" + r, tag="mt")
                    nc.sync.dma_start(
                        mt_t[:], MT[c * 128:(c + 1) * 128, r0:r0 + RG])
        return

    with tc.tile_pool(name="qkv" + r, bufs=1) as qkvpool:
        # V_ext: [m-part, chunk, 64 V dims + ones col (+pad)] in bf16
        v_sb = qkvpool.tile([128, N_MC, 66], bf16, name="v_sb" + r)
        nc.vector.memset(v_sb[:, :, 64:65], 1.0)
        qt_sb = qkvpool.tile([HD, N], f32r, name="qt_sb" + r)
        kt_sb = qkvpool.tile([HD, N], f32r, name="kt_sb" + r)

        # ---- Phase 1: Q^T, K^T, V from X^T ----
        with (
            tc.tile_pool(name="xt" + r, bufs=1) as xtpool,
            tc.tile_pool(name="qkvps" + r, bufs=2, space="PSUM") as qkvps,
        ):
            xts = []
            for c in range(4):
                xt_c = xtpool.tile([128, N], f32r, name=f"xt_{c}" + r, tag=f"xt{c}")
                nc.sync.dma_start(xt_c[:], XT[c * 128:(c + 1) * 128, :])
                xts.append(xt_c)
            for wt, dst in ((wq_sb, qt_sb), (wk_sb, kt_sb)):
                for t in range(N // NSPLIT):
                    ps = qkvps.tile([HD, NSPLIT], f32, name="ps_qk" + r, tag="qk")
                    for c in range(4):
                        nc.tensor.matmul(
                            ps[:],
                            lhsT=wt[:, c, :],
                            rhs=xts[c][:, t * NSPLIT:(t + 1) * NSPLIT],
                            start=(c == 0), stop=(c == 3))
                    nc.vector.tensor_copy(dst[:, t * NSPLIT:(t + 1) * NSPLIT], ps[:])
            for m in range(N_MC):
                psv = qkvps.tile([128, HD], f32, name="ps_v" + r, tag="v")
                for c in range(4):
                    nc.tensor.matmul(
                        psv[:],
                        lhsT=xts[c][:, m * 128:(m + 1) * 128],
                        rhs=wv_sb[:, c, :],
                        start=(c == 0), stop=(c == 3))
                nc.vector.tensor_copy(v_sb[:, m, 0:HD], psv[:])

        # ---- Phase 2: attention main loop ----
        with (
            tc.tile_pool(name="mpool" + r, bufs=4) as mpool,
            tc.tile_pool(name="ppool" + r, bufs=4) as ppool,
            tc.tile_pool(name="fpool" + r, bufs=2) as fpool,
            tc.tile_pool(name="opool" + r, bufs=3) as opool,
            tc.tile_pool(name="eps" + r, bufs=3, space="PSUM") as eps,
            tc.tile_pool(name="accps" + r, bufs=1, space="PSUM") as accps,
        ):
            LAG = 2  # PE software-pipeline depth: PV_c emitted after E_{c+LAG}
            for g in range(N_RG):
                r0 = g * RG
                acc = accps.tile([HD + 1, RG], f32, name="acc" + r, tag="acc")
                pts = {}
                for cc in range(N_MC + LAG):
                    if cc < N_MC:
                        c = cc
                        if variant != "nomaskdma":
                            mt_t = mpool.tile([128, RG], bf16, name="mt_t" + r,
                                              tag="mt")
                            eng = nc.sync if (c % 2 == 0) else nc.gpsimd
                            eng.dma_start(
                                mt_t[:], MT[c * 128:(c + 1) * 128, r0:r0 + RG])
                        elif c == 0 and g == 0:
                            mt_t = mpool.tile([128, RG], bf16, name="mt_t" + r,
                                              tag="mt", bufs=1)
                            nc.vector.memset(mt_t[:], 1.0)
                        es = eps.tile([128, RG], f32, name="es" + r, tag="es")
                        for s in range(RG // NSPLIT):
                            nc.tensor.matmul(
                                es[:, s * NSPLIT:(s + 1) * NSPLIT],
                                lhsT=kt_sb[:, c * 128:(c + 1) * 128],
                                rhs=qt_sb[:, r0 + s * NSPLIT:
                                          r0 + (s + 1) * NSPLIT],
                                start=True, stop=True)
                        p_t = ppool.tile([128, RG], bf16, name="p_t" + r,
                                         tag="p")
                        nc.scalar.activation(p_t[:], es[:], AF.Exp,
                                             bias=zeros_p[:], scale=SCALE)
                        nc.vector.tensor_mul(p_t[:], p_t[:], mt_t[:])
                        pts[c] = p_t
                    if cc >= LAG:
                        c = cc - LAG
                        p_t = pts.pop(c)
                        for s in range(RG // NSPLIT):
                            nc.tensor.matmul(
                                acc[:, s * NSPLIT:(s + 1) * NSPLIT],
                                lhsT=v_sb[:, c, 0:HD + 1],
                                rhs=p_t[:, s * NSPLIT:(s + 1) * NSPLIT],
                                start=(c == 0), stop=(c == N_MC - 1),
                                skip_group_check=True)

                # ---- finalize rowgroup: W_O projection + normalization ----
                ot_sb = fpool.tile([HD, RG], f32r, name="ot_sb" + r, tag="ot")
                nc.vector.tensor_copy(ot_sb[:], acc[0:HD, :])
                s_sb = fpool.tile([1, RG], f32, name="s_sb" + r, tag="s")
                nc.scalar.copy(s_sb[:], acc[HD:HD + 1, :])
                st_ps = eps.tile([128, RG // 128], f32, name="st_ps" + r, tag="es")
                for j in range(RG // 128):
                    nc.tensor.transpose(
                        st_ps[:, j:j + 1],
                        s_sb[0:1, j * 128:(j + 1) * 128],
                        ident11[:])
                rt_sb = fpool.tile([128, RG // 128], f32, name="rt_sb" + r, tag="rt")
                nc.vector.reciprocal(rt_sb[:], st_ps[:])
                for j in range(RG // 128):
                    pso = eps.tile([128, F_OUT], f32, name="pso" + r, tag="es")
                    nc.tensor.matmul(
                        pso[:],
                        lhsT=ot_sb[:, j * 128:(j + 1) * 128],
                        rhs=wo_sb[:],
                        start=True, stop=True)
                    out_sb = opool.tile([128, F_OUT], f32, name="out_sb" + r,
                                        tag="out")
                    nc.vector.tensor_scalar_mul(out_sb[:], pso[:],
                                                rt_sb[:, j:j + 1])
                    nc.sync.dma_start(
                        OUT[r0 + j * 128:r0 + (j + 1) * 128, :], out_sb[:])


def _shard_inputs(X, mask, W_Q, W_K, W_V, W_O):
    """Per-core input dicts (host-side layout prep)."""
    in_maps = []
    for h in range(H):
        xt = np.ascontiguousarray(X[h].T)                      # [512, 4096] f32
        # mask[h].T as bf16 bits: 1 -> 0x3F80 (bf16 1.0), 0 -> 0
        m16 = mask[h].view(np.uint16)[:, 0::2]                 # low half of i32
        mt = (m16.T * np.uint16(0x3F80)).view(BF16)            # [4096, 4096] bf16
        wq = np.ascontiguousarray(
            W_Q[h].T.reshape(4, 128, HD).transpose(1, 0, 2))   # [128, 4, 64]
        wk = np.ascontiguousarray(
            W_K[h].T.reshape(4, 128, HD).transpose(1, 0, 2))
        wv = np.ascontiguousarray(
            W_V[h].T.reshape(4, 128, HD).transpose(1, 0, 2))
        wo = np.ascontiguousarray(W_O[:, h * HD:(h + 1) * HD].T)  # [64, 512]
        in_maps.append({"xt": xt, "mt": mt, "wq": wq, "wk": wk,
                        "wv": wv, "wo": wo})
    return in_maps


def kernel(X, mask, W_Q, W_K, W_V, W_O):
    from concourse.bass_utils import run_bass_kernel_spmd
    nc = _build_program(repeat=1)
    in_maps = _shard_inputs(X, mask, W_Q, W_K, W_V, W_O)
    res = run_bass_kernel_spmd(nc, in_maps, list(range(N_CORES)))
    out = np.zeros((N, F_OUT), np.float64)
    for h in range(H):
        out += res.results[h]["out"].astype(np.float64)
    return out.astype(np.float32)



# revision 24
# speedup vs baseline: 1.2520x; 1.2520x over previous
"""Multi-head sparse attention on 8 NeuronCores (Trainium2, Bass/Tile).

Head-parallel sharding: core h owns head h (H == n_cores == 8).
Each core computes its head's attention output and the partial final
projection through its W_O column slice; the host sums the 8 partials.

Math note: softmax rows are never fully masked (random 0/1 mask) and
E = QK^T/8 has |E| <~ 6, so softmax is computed WITHOUT max subtraction:
P = exp(E) * mask, out = (P @ V) / rowsum(P).  rowsum is obtained by
appending a ones-column to V, and the normalization is deferred until
after the W_O projection (a per-row scalar).

v2 layout: everything bf16 on the wire (X^T, W, mask, OUT), Q^T/K^T
computed in one packed matmul chain, mask DMA'd 4 chunks per issue,
output stored once per rowgroup, normalization multiply on gpsimd.
"""

import numpy as np
import ml_dtypes

H, N, F_IN, HD, F_OUT = 8, 4096, 512, 64, 512
N_CORES = 8
RG = 1024            # query-row group processed per PSUM accumulator
N_RG = N // RG       # 4
MC = 128             # key/m chunk (partition dim)
N_MC = N // MC       # 32
NSPLIT = 512         # matmul moving-operand free size
MB = 4               # mask chunks per DMA batch
NB = N_MC // MB      # mask batches per rowgroup (8)
BF16 = ml_dtypes.bfloat16

_PROGRAM_CACHE = {}


def _build_program(repeat=1, timing=False, variant="full"):
    """Build + compile the Bass/Tile program (same SPMD program for all cores).

    timing=True builds a benchmark variant: inputs live in internal DRAM
    (initialized on-device), the body runs `repeat` times inside a hardware
    For_i loop, and only a tiny checksum output is external.  Differencing
    the wall time of two repeat counts isolates the per-iteration HW time.
    """
    key = (repeat, timing, variant)
    if key in _PROGRAM_CACHE:
        return _PROGRAM_CACHE[key]

    import concourse.bacc as bacc
    import concourse.tile as tile
    import concourse.mybir as mybir

    f32 = mybir.dt.float32
    f32r = mybir.dt.float32r
    bf16 = mybir.dt.bfloat16

    nc = bacc.Bacc("TRN2", target_bir_lowering=False, debug=False,
                   num_devices=N_CORES)

    kind_in = {} if timing else {"kind": "ExternalInput"}
    XT = nc.dram_tensor("xt", [F_IN, N], bf16, **kind_in).ap()
    MT = nc.dram_tensor("mt", [N_RG, NB, MB * 128, RG], bf16,
                        **kind_in).ap()
    WQK = nc.dram_tensor("wqk", [128, 4, 128], bf16, **kind_in).ap()
    WV = nc.dram_tensor("wv", [128, 4, HD], bf16, **kind_in).ap()
    WO = nc.dram_tensor("wo", [HD, F_OUT], bf16, **kind_in).ap()
    if not timing:
        OUT = nc.dram_tensor("out", [N, F_OUT], bf16,
                             kind="ExternalOutput").ap()
    else:
        OUT = nc.dram_tensor("out", [N, F_OUT], bf16).ap()
        DUMMY = nc.dram_tensor("dumin", [1, 8], f32, kind="ExternalInput").ap()
        CHK = nc.dram_tensor("chk", [128, F_OUT], bf16,
                             kind="ExternalOutput").ap()

    SCALE = float(1.0 / np.sqrt(HD))

    with tile.TileContext(nc) as tc:
        with (
            tc.tile_pool(name="consts", bufs=1) as consts,
            tc.tile_pool(name="wpool", bufs=1) as wpool,
        ):
            ident11 = consts.tile([1, 1], f32)
            nc.vector.memset(ident11[:], 1.0)
            zeros_p = consts.tile([128, 1], f32)
            nc.vector.memset(zeros_p[:], 0.0)

            wqk_sb = wpool.tile([128, 4, 128], bf16)
            wv_sb = wpool.tile([128, 4, HD], bf16)
            wo_sb = wpool.tile([HD, F_OUT], bf16)

            if timing:
                # on-device init of internal DRAM inputs (runs once)
                with tc.tile_pool(name="init", bufs=1) as initp:
                    mrow = initp.tile([128, N], bf16)
                    nc.vector.memset(mrow[:], 1.0)
                    MTf = MT.rearrange("g b p q -> (g b p) q")
                    for c in range(N * N // (128 * RG)):
                        nc.sync.dma_start(MTf[c * 128:(c + 1) * 128, :],
                                          mrow[:, 0:RG])
                    xrow = initp.tile([128, N], bf16)
                    nc.vector.memset(xrow[:], 0.015625)
                    for c in range(4):
                        nc.sync.dma_start(XT[c * 128:(c + 1) * 128, :], xrow[:])
                    wrow = initp.tile([128, 4 * 128], bf16)
                    nc.vector.memset(wrow[:], 0.03125)
                    nc.sync.dma_start(WQK.rearrange("p c d -> p (c d)"),
                                      wrow[:])
                    nc.sync.dma_start(WV.rearrange("p c d -> p (c d)"),
                                      wrow[:, 0:4 * HD])
                    worow = initp.tile([HD, F_OUT], bf16)
                    nc.vector.memset(worow[:], 0.03125)
                    nc.sync.dma_start(WO[:], worow[:])

            nc.sync.dma_start(wqk_sb[:], WQK[:])
            nc.sync.dma_start(wv_sb[:], WV[:])
            nc.sync.dma_start(wo_sb[:], WO[:])

            if timing and repeat > 1:
                with tc.For_i(0, repeat, 1):
                    _one_pass(nc, tc, mybir, XT, MT, OUT,
                              wqk_sb, wv_sb, wo_sb, ident11, zeros_p,
                              SCALE, 0, variant)
            else:
                for rep in range(repeat):
                    _one_pass(nc, tc, mybir, XT, MT, OUT,
                              wqk_sb, wv_sb, wo_sb, ident11, zeros_p,
                              SCALE, rep, variant)

            if timing:
                with tc.tile_pool(name="chkp", bufs=1) as chkp:
                    chk_sb = chkp.tile([128, F_OUT], bf16)
                    nc.sync.dma_start(chk_sb[:], OUT[0:128, :])
                    nc.sync.dma_start(CHK[:], chk_sb[:])

    nc.compile()
    _PROGRAM_CACHE[key] = nc
    return nc


def _pool_activation(nc, mybir, out, in_, func, bias, scale):
    """InstActivation issued on the Pool (gpsimd) engine."""
    eng = nc.gpsimd
    ins = [eng.lower_ap(in_), eng.lower_ap(bias),
           mybir.ImmediateValue(dtype=mybir.dt.float32, value=scale),
           mybir.ImmediateValue(dtype=mybir.dt.float32, value=0.0)]
    return eng.add_instruction(
        mybir.InstActivation(
            name=nc.get_next_instruction_name(),
            func=func,
            ins=ins,
            outs=[eng.lower_ap(out)],
        ))


# number of exp tiles (of 32 chunks/rowgroup) offloaded to gpsimd
POOL_EXP = 0


def _one_pass(nc, tc, mybir, XT, MT, OUT,
              wqk_sb, wv_sb, wo_sb, ident11, zeros_p, SCALE, rep,
              variant="full"):
    f32 = mybir.dt.float32
    f32r = mybir.dt.float32r
    bf16 = mybir.dt.bfloat16
    AF = mybir.ActivationFunctionType
    r = f"_r{rep}"

    with (
        tc.tile_pool(name="qkv" + r, bufs=1) as qkvpool,
        tc.tile_pool(name="mpool" + r, bufs=4) as mpool,
        tc.tile_pool(name="ppool" + r, bufs=5) as ppool,
        tc.tile_pool(name="fpool" + r, bufs=2) as fpool,
        tc.tile_pool(name="opool" + r, bufs=2) as opool,
    ):
        # V_ext: [m-part, chunk, 64 V dims + ones col (+pad)] in bf16
        v_sb = qkvpool.tile([128, N_MC, 66], bf16, name="v_sb" + r)
        nc.vector.memset(v_sb[:, :, 64:65], 1.0)
        qt_sb = qkvpool.tile([HD, N], bf16, name="qt_sb" + r)
        kt_sb = qkvpool.tile([HD, N], bf16, name="kt_sb" + r)

        def mask_batch(g, b):
            """Issue one batched mask DMA: chunks 4b..4b+3, rowgroup g."""
            mt4 = mpool.tile([128, MB, RG], bf16, name="mt4" + r, tag="mt")
            eng = nc.sync if (b % 2 == 0) else nc.gpsimd
            eng.dma_start(
                mt4[:],
                MT[g, b, :, :].rearrange("(k p) q -> p k q", k=MB))
            return mt4

        # ---- Phase 1: [Q^T;K^T] packed and V from X^T ----
        # c-outer accumulation: all 8 QK psum tiles live at once (8 banks)
        # so PE work starts as soon as xt_0 arrives instead of after all 4.
        mt_tiles = {}
        NT = N // NSPLIT
        with (
            tc.tile_pool(name="xt" + r, bufs=1) as xtpool,
            tc.tile_pool(name="qkvps" + r, bufs=8, space="PSUM") as qkvps,
        ):
            # xt loads first (split across both DMA queues), then mask
            # prefetch behind them
            xts = []
            for c in range(4):
                xt_c = xtpool.tile([128, N], bf16, name=f"xt_{c}" + r,
                                   tag=f"xt{c}")
                eng = nc.sync if (c % 2 == 0) else nc.gpsimd
                eng.dma_start(xt_c[:], XT[c * 128:(c + 1) * 128, :])
                xts.append(xt_c)
            if variant != "nomaskdma":
                for b in range(3):
                    mt_tiles[(0, b)] = mask_batch(0, b)
            else:
                mt4 = mpool.tile([128, MB, RG], bf16, name="mt4" + r,
                                 tag="mt", bufs=1)
                nc.vector.memset(mt4[:], 1.0)
                for g in range(N_RG):
                    for b in range(NB):
                        mt_tiles[(g, b)] = mt4
            pss = [qkvps.tile([128, NSPLIT], f32, name=f"ps_qk{t}" + r,
                              tag="qk")
                   for t in range(NT)]
            for c in range(4):
                for t in range(NT):
                    nc.tensor.matmul(
                        pss[t][:],
                        lhsT=wqk_sb[:, c, :],
                        rhs=xts[c][:, t * NSPLIT:(t + 1) * NSPLIT],
                        start=(c == 0), stop=(c == 3))
            for t in range(NT):
                nc.vector.tensor_copy(qt_sb[:, t * NSPLIT:(t + 1) * NSPLIT],
                                      pss[t][0:HD, :])
                nc.vector.tensor_copy(kt_sb[:, t * NSPLIT:(t + 1) * NSPLIT],
                                      pss[t][HD:128, :])
            for m4 in range(N_MC // 4):
                psv = qkvps.tile([128, 4, HD], f32, name="ps_v" + r, tag="qk")
                for i in range(4):
                    m = m4 * 4 + i
                    for c in range(4):
                        nc.tensor.matmul(
                            psv[:, i, :],
                            lhsT=xts[c][:, m * 128:(m + 1) * 128],
                            rhs=wv_sb[:, c, :],
                            start=(c == 0), stop=(c == 3))
                nc.vector.tensor_copy(v_sb[:, m4 * 4:(m4 + 1) * 4, 0:HD],
                                      psv[:])

        # ---- Phase 2: attention main loop ----
        ctx2 = tc.tile_pool(name="eps" + r, bufs=2, space="PSUM")
        eps = ctx2.__enter__()
        ctx3 = tc.tile_pool(name="accps" + r, bufs=2, space="PSUM")
        accps = ctx3.__enter__()
        LAG = 3  # PE software-pipeline depth: PV_c emitted after E_{c+LAG}
        NJ = RG // 128

        def finalize_steps(g, acc):
            """Per-rowgroup epilogue as a list of thunks; interleaved into
            the next rowgroup's chunk loop so the PE/Act pipeline never
            drains at rowgroup boundaries."""
            r0 = g * RG
            st = {}

            def s_copies():
                st["ot"] = fpool.tile([HD, RG], bf16, name="ot_sb" + r,
                                      tag="ot")
                nc.vector.tensor_copy(st["ot"][:], acc[0:HD, :])
                st["s"] = fpool.tile([1, RG], f32, name="s_sb" + r, tag="s")
                nc.vector.tensor_copy(st["s"][:], acc[HD:HD + 1, :])

            def s_recip():
                stp = eps.tile([128, NJ], f32, name="st_ps" + r, tag="es")
                for j in range(NJ):
                    nc.tensor.transpose(
                        stp[:, j:j + 1],
                        st["s"][0:1, j * 128:(j + 1) * 128],
                        ident11[:])
                st["rt"] = fpool.tile([128, NJ], f32, name="rt_sb" + r,
                                      tag="rt")
                nc.vector.reciprocal(st["rt"][:], stp[:])
                st["out"] = opool.tile([128, NJ, F_OUT], bf16,
                                       name="out_t" + r, tag="out")

            def s_wo(j):
                def f():
                    pso = eps.tile([128, F_OUT], f32, name="pso" + r,
                                   tag="es")
                    nc.tensor.matmul(
                        pso[:],
                        lhsT=st["ot"][:, j * 128:(j + 1) * 128],
                        rhs=wo_sb[:],
                        start=True, stop=True)
                    nc.vector.tensor_scalar_mul(st["out"][:, j, :], pso[:],
                                                st["rt"][:, j:j + 1])
                return f

            def s_store():
                nc.sync.dma_start(
                    OUT[r0:r0 + RG, :].rearrange("(j p) f -> p j f", j=NJ),
                    st["out"][:])

            return ([s_copies, s_recip]
                    + [s_wo(j) for j in range(NJ)] + [s_store])

        # Flat pipeline over all (g, c) chunks: E/exp/mask lead, PV lags
        # LAG chunks behind (crossing rowgroup boundaries), the previous
        # rowgroup's finalize interleaves one step per slot.
        NTOT = N_RG * N_MC
        accs = {}
        pts = {}
        pending = []
        for k in range(NTOT + LAG):
            if k < NTOT:
                g, c = divmod(k, N_MC)
                r0 = g * RG
                b = c // MB
                if variant != "nomaskdma":
                    # prefetch ~3 batches ahead
                    nxt = (g, b + 3) if b + 3 < NB else (g + 1, b + 3 - NB)
                    if c % MB == 0 and nxt[0] < N_RG and nxt not in mt_tiles:
                        mt_tiles[nxt] = mask_batch(*nxt)
                    if (g, b) not in mt_tiles:
                        mt_tiles[(g, b)] = mask_batch(g, b)
                mt4 = mt_tiles[(g, b)]
                es = eps.tile([128, RG], f32, name="es" + r, tag="es")
                for s in range(RG // NSPLIT):
                    nc.tensor.matmul(
                        es[:, s * NSPLIT:(s + 1) * NSPLIT],
                        lhsT=kt_sb[:, c * 128:(c + 1) * 128],
                        rhs=qt_sb[:, r0 + s * NSPLIT:
                                  r0 + (s + 1) * NSPLIT],
                        start=True, stop=True)
                p_t = ppool.tile([128, RG], bf16, name="p_t" + r, tag="p")
                nc.scalar.activation(p_t[:], es[:], AF.Exp,
                                     bias=zeros_p[:], scale=SCALE)
                nc.vector.tensor_mul(p_t[:], p_t[:], mt4[:, c % MB, :])
                pts[k] = p_t
            if k >= LAG:
                kv = k - LAG
                gv, cv = divmod(kv, N_MC)
                if cv == 0:
                    accs[gv] = accps.tile([HD + 1, RG], f32, name="acc" + r,
                                          tag="acc")
                p_t = pts.pop(kv)
                for s in range(RG // NSPLIT):
                    nc.tensor.matmul(
                        accs[gv][:, s * NSPLIT:(s + 1) * NSPLIT],
                        lhsT=v_sb[:, cv, 0:HD + 1],
                        rhs=p_t[:, s * NSPLIT:(s + 1) * NSPLIT],
                        start=(cv == 0), stop=(cv == N_MC - 1),
                        skip_group_check=True)
                if cv == N_MC - 1:
                    for f in pending:   # rare leftover (shouldn't happen)
                        f()
                    pending = finalize_steps(gv, accs.pop(gv))
            if pending:
                pending.pop(0)()
        for f in pending:
            f()
        ctx3.__exit__(None, None, None)
        ctx2.__exit__(None, None, None)


def _shard_inputs(X, mask, W_Q, W_K, W_V, W_O):
    """Per-core input dicts (host-side layout prep)."""
    in_maps = []
    for h in range(H):
        xt = X[h].T.astype(BF16)                               # [512, 4096]
        # mask[h].T as bf16 bits: 1 -> 0x3F80 (bf16 1.0), 0 -> 0
        m16 = mask[h].view(np.uint16)[:, 0::2]                 # low half of i32
        mt = (m16.T * np.uint16(0x3F80)).view(BF16)            # [4096, 4096]
        # tile: [keys, queries] -> [g, b, 512, 1024] contiguous per batch
        mt = np.ascontiguousarray(
            mt.reshape(NB, MB * 128, N_RG, RG).transpose(2, 0, 1, 3))
        wq = W_Q[h].T.reshape(4, 128, HD).transpose(1, 0, 2)   # [128, 4, 64]
        wk = W_K[h].T.reshape(4, 128, HD).transpose(1, 0, 2)
        wqk = np.concatenate([wq, wk], axis=2).astype(BF16)    # [128, 4, 128]
        wv = np.ascontiguousarray(
            W_V[h].T.reshape(4, 128, HD).transpose(1, 0, 2)).astype(BF16)
        wo = W_O[:, h * HD:(h + 1) * HD].T.astype(BF16)        # [64, 512]
        in_maps.append({"xt": xt, "mt": mt, "wqk": wqk,
                        "wv": wv, "wo": wo})
    return in_maps


def kernel(X, mask, W_Q, W_K, W_V, W_O):
    from concourse.bass_utils import run_bass_kernel_spmd
    nc = _build_program(repeat=1)
    in_maps = _shard_inputs(X, mask, W_Q, W_K, W_V, W_O)
    res = run_bass_kernel_spmd(nc, in_maps, list(range(N_CORES)))
    out = np.zeros((N, F_OUT), np.float64)
    for h in range(H):
        out += res.results[h]["out"].astype(np.float64)
    return out.astype(np.float32)


# revision 31
# speedup vs baseline: 1.2523x; 1.0002x over previous
"""Multi-head sparse attention on 8 NeuronCores (Trainium2, Bass/Tile).

Head-parallel sharding: core h owns head h (H == n_cores == 8).
Each core computes its head's attention output and the partial final
projection through its W_O column slice; the host sums the 8 partials.

Math note: softmax rows are never fully masked (random 0/1 mask) and
E = QK^T/8 has |E| <~ 6, so softmax is computed WITHOUT max subtraction:
P = exp(E) * mask, out = (P @ V) / rowsum(P).  rowsum is obtained by
appending a ones-column to V, and the normalization is deferred until
after the W_O projection (a per-row scalar).

v2 layout: everything bf16 on the wire (X^T, W, mask, OUT), Q^T/K^T
computed in one packed matmul chain, mask DMA'd 4 chunks per issue,
output stored once per rowgroup, normalization multiply on gpsimd.
"""

import numpy as np
import ml_dtypes

H, N, F_IN, HD, F_OUT = 8, 4096, 512, 64, 512
N_CORES = 8
RG = 1024            # query-row group processed per PSUM accumulator
N_RG = N // RG       # 4
MC = 128             # key/m chunk (partition dim)
N_MC = N // MC       # 32
NSPLIT = 512         # matmul moving-operand free size
MB = 4               # mask chunks per DMA batch
NB = N_MC // MB      # mask batches per rowgroup (8)
BF16 = ml_dtypes.bfloat16

_PROGRAM_CACHE = {}


def _build_program(repeat=1, timing=False, variant="full"):
    """Build + compile the Bass/Tile program (same SPMD program for all cores).

    timing=True builds a benchmark variant: inputs live in internal DRAM
    (initialized on-device), the body runs `repeat` times inside a hardware
    For_i loop, and only a tiny checksum output is external.  Differencing
    the wall time of two repeat counts isolates the per-iteration HW time.
    """
    key = (repeat, timing, variant)
    if key in _PROGRAM_CACHE:
        return _PROGRAM_CACHE[key]

    import concourse.bacc as bacc
    import concourse.tile as tile
    import concourse.mybir as mybir

    f32 = mybir.dt.float32
    f32r = mybir.dt.float32r
    bf16 = mybir.dt.bfloat16

    nc = bacc.Bacc("TRN2", target_bir_lowering=False, debug=False,
                   num_devices=N_CORES)

    kind_in = {} if timing else {"kind": "ExternalInput"}
    XT = nc.dram_tensor("xt", [F_IN, N], bf16, **kind_in).ap()
    MT = nc.dram_tensor("mt", [N_RG, NB, MB * 128, RG], bf16,
                        **kind_in).ap()
    WQK = nc.dram_tensor("wqk", [128, 4, 128], bf16, **kind_in).ap()
    WV = nc.dram_tensor("wv", [128, 4, HD], bf16, **kind_in).ap()
    WO = nc.dram_tensor("wo", [HD, F_OUT], bf16, **kind_in).ap()
    if not timing:
        OUT = nc.dram_tensor("out", [N, F_OUT], bf16,
                             kind="ExternalOutput").ap()
    else:
        OUT = nc.dram_tensor("out", [N, F_OUT], bf16).ap()
        DUMMY = nc.dram_tensor("dumin", [1, 8], f32, kind="ExternalInput").ap()
        CHK = nc.dram_tensor("chk", [128, F_OUT], bf16,
                             kind="ExternalOutput").ap()

    SCALE = float(1.0 / np.sqrt(HD))

    with tile.TileContext(nc) as tc:
        with (
            tc.tile_pool(name="consts", bufs=1) as consts,
            tc.tile_pool(name="wpool", bufs=1) as wpool,
        ):
            ident11 = consts.tile([1, 1], f32)
            nc.vector.memset(ident11[:], 1.0)
            zeros_p = consts.tile([128, 1], f32)
            nc.vector.memset(zeros_p[:], 0.0)
            m30_p = consts.tile([128, 1], f32)
            nc.vector.memset(m30_p[:], -30.0)

            wqk_sb = wpool.tile([128, 4, 128], bf16)
            wv_sb = wpool.tile([128, 4, HD], bf16)
            wo_sb = wpool.tile([HD, F_OUT], bf16)

            if timing:
                # on-device init of internal DRAM inputs (runs once)
                with tc.tile_pool(name="init", bufs=1) as initp:
                    mrow = initp.tile([128, N], bf16)
                    nc.vector.memset(mrow[:], 1.0)
                    MTf = MT.rearrange("g b p q -> (g b p) q")
                    for c in range(N * N // (128 * RG)):
                        nc.sync.dma_start(MTf[c * 128:(c + 1) * 128, :],
                                          mrow[:, 0:RG])
                    xrow = initp.tile([128, N], bf16)
                    nc.vector.memset(xrow[:], 0.015625)
                    for c in range(4):
                        nc.sync.dma_start(XT[c * 128:(c + 1) * 128, :], xrow[:])
                    wrow = initp.tile([128, 4 * 128], bf16)
                    nc.vector.memset(wrow[:], 0.03125)
                    nc.sync.dma_start(WQK.rearrange("p c d -> p (c d)"),
                                      wrow[:])
                    nc.sync.dma_start(WV.rearrange("p c d -> p (c d)"),
                                      wrow[:, 0:4 * HD])
                    worow = initp.tile([HD, F_OUT], bf16)
                    nc.vector.memset(worow[:], 0.03125)
                    nc.sync.dma_start(WO[:], worow[:])

            nc.sync.dma_start(wqk_sb[:], WQK[:])
            nc.sync.dma_start(wv_sb[:], WV[:])
            nc.sync.dma_start(wo_sb[:], WO[:])

            if timing and repeat > 1:
                with tc.For_i(0, repeat, 1):
                    _one_pass(nc, tc, mybir, XT, MT, OUT,
                              wqk_sb, wv_sb, wo_sb, ident11, zeros_p,
                              m30_p, SCALE, 0, variant)
            else:
                for rep in range(repeat):
                    _one_pass(nc, tc, mybir, XT, MT, OUT,
                              wqk_sb, wv_sb, wo_sb, ident11, zeros_p,
                              m30_p, SCALE, rep, variant)

            if timing:
                with tc.tile_pool(name="chkp", bufs=1) as chkp:
                    chk_sb = chkp.tile([128, F_OUT], bf16)
                    nc.sync.dma_start(chk_sb[:], OUT[0:128, :])
                    nc.sync.dma_start(CHK[:], chk_sb[:])

    nc.compile()
    _PROGRAM_CACHE[key] = nc
    return nc


def _pool_activation(nc, mybir, out, in_, func, bias, scale):
    """InstActivation issued on the Pool (gpsimd) engine."""
    eng = nc.gpsimd
    ins = [eng.lower_ap(in_), eng.lower_ap(bias),
           mybir.ImmediateValue(dtype=mybir.dt.float32, value=scale),
           mybir.ImmediateValue(dtype=mybir.dt.float32, value=0.0)]
    return eng.add_instruction(
        mybir.InstActivation(
            name=nc.get_next_instruction_name(),
            func=func,
            ins=ins,
            outs=[eng.lower_ap(out)],
        ))


# number of exp tiles (of 32 chunks/rowgroup) offloaded to gpsimd
POOL_EXP = 0


def _one_pass(nc, tc, mybir, XT, MT, OUT,
              wqk_sb, wv_sb, wo_sb, ident11, zeros_p, m30_p, SCALE, rep,
              variant="full"):
    f32 = mybir.dt.float32
    f32r = mybir.dt.float32r
    bf16 = mybir.dt.bfloat16
    AF = mybir.ActivationFunctionType
    r = f"_r{rep}"

    with (
        tc.tile_pool(name="qkv" + r, bufs=1) as qkvpool,
        tc.tile_pool(name="mpool" + r, bufs=4) as mpool,
        tc.tile_pool(name="ppool" + r, bufs=5) as ppool,
        tc.tile_pool(name="fpool" + r, bufs=2) as fpool,
        tc.tile_pool(name="opool" + r, bufs=2) as opool,
        tc.tile_pool(name="tpool" + r, bufs=2) as tpool,
    ):
        if variant == "dmaonly":
            dout = opool.tile([128, RG // 128, F_OUT], bf16,
                              name="dout" + r, tag="out")
            nc.vector.memset(dout[:], 0.0)
            for c in range(4):
                xt_c = tpool.tile([128, N], bf16, name=f"dxt_{c}" + r,
                                  tag="tmp")
                eng = nc.sync if (c % 2 == 0) else nc.gpsimd
                eng.dma_start(xt_c[:], XT[c * 128:(c + 1) * 128, :])
            for g in range(N_RG):
                for b in range(NB):
                    mt4 = mpool.tile([128, MB, RG], bf16, name="mt4" + r,
                                     tag="mt")
                    eng = nc.sync if (b % 2 == 0) else nc.gpsimd
                    eng.dma_start(
                        mt4[:],
                        MT[g, b, :, :].rearrange("(k p) q -> p k q", k=MB))
                nc.sync.dma_start(
                    OUT[g * RG:(g + 1) * RG, :].rearrange(
                        "(j p) f -> p j f", j=RG // 128),
                    dout[:])
            return
        # V_ext: [m-part, chunk, 64 V dims + ones col (+pad)] in bf16
        v_sb = qkvpool.tile([128, N_MC, 66], bf16, name="v_sb" + r)
        nc.vector.memset(v_sb[:, :, 64:65], 1.0)
        qt_sb = qkvpool.tile([HD, N], bf16, name="qt_sb" + r)
        kt_sb = qkvpool.tile([HD, N], bf16, name="kt_sb" + r)

        def mask_batch(g, b):
            """Issue one batched mask DMA: chunks 4b..4b+3, rowgroup g."""
            mt4 = mpool.tile([128, MB, RG], bf16, name="mt4" + r, tag="mt")
            eng = nc.sync if (b % 2 == 0) else nc.gpsimd
            eng.dma_start(
                mt4[:],
                MT[g, b, :, :].rearrange("(k p) q -> p k q", k=MB))
            return mt4

        # ---- Phase 1: [Q^T;K^T] packed and V from X^T ----
        # c-outer accumulation: all 8 QK psum tiles live at once (8 banks)
        # so PE work starts as soon as xt_0 arrives instead of after all 4.
        mt_tiles = {}
        NT = N // NSPLIT
        with (
            tc.tile_pool(name="xt" + r, bufs=1) as xtpool,
            tc.tile_pool(name="qkvps" + r, bufs=4, space="PSUM") as qkvps,
        ):
            # xt loads first (split across both DMA queues), then mask
            # prefetch behind them
            xts = []
            for c in range(4):
                xt_c = xtpool.tile([128, N], bf16, name=f"xt_{c}" + r,
                                   tag=f"xt{c}")
                eng = nc.sync if (c % 2 == 0) else nc.gpsimd
                eng.dma_start(xt_c[:], XT[c * 128:(c + 1) * 128, :])
                xts.append(xt_c)
            if variant != "nomaskdma":
                for b in range(3):
                    mt_tiles[(0, b)] = mask_batch(0, b)
            else:
                mt4 = mpool.tile([128, MB, RG], bf16, name="mt4" + r,
                                 tag="mt", bufs=1)
                nc.vector.memset(mt4[:], 1.0)
                for g in range(N_RG):
                    for b in range(NB):
                        mt_tiles[(g, b)] = mt4
            for t in range(NT):
                ps = qkvps.tile([128, NSPLIT], f32, name="ps_qk" + r,
                                tag="qk")
                for c in range(4):
                    nc.tensor.matmul(
                        ps[:],
                        lhsT=wqk_sb[:, c, :],
                        rhs=xts[c][:, t * NSPLIT:(t + 1) * NSPLIT],
                        start=(c == 0), stop=(c == 3))
                nc.vector.tensor_copy(qt_sb[:, t * NSPLIT:(t + 1) * NSPLIT],
                                      ps[0:HD, :])
                nc.vector.tensor_copy(kt_sb[:, t * NSPLIT:(t + 1) * NSPLIT],
                                      ps[HD:128, :])
            for m4 in range(N_MC // 4):
                psv = qkvps.tile([128, 4, HD], f32, name="ps_v" + r, tag="qk")
                for i in range(4):
                    m = m4 * 4 + i
                    for c in range(4):
                        nc.tensor.matmul(
                            psv[:, i, :],
                            lhsT=xts[c][:, m * 128:(m + 1) * 128],
                            rhs=wv_sb[:, c, :],
                            start=(c == 0), stop=(c == 3))
                nc.vector.tensor_copy(v_sb[:, m4 * 4:(m4 + 1) * 4, 0:HD],
                                      psv[:])

        # ---- Phase 2: attention main loop ----
        ctx2 = tc.tile_pool(name="eps" + r, bufs=3, space="PSUM")
        eps = ctx2.__enter__()
        ctx3 = tc.tile_pool(name="accps" + r, bufs=1, space="PSUM")
        accps = ctx3.__enter__()
        LAG = 3  # PE software-pipeline depth: PV_c emitted after E_{c+LAG}
        NJ = RG // 128

        def finalize_steps(g, acc):
            """Per-rowgroup epilogue as a list of thunks; interleaved into
            the next rowgroup's chunk loop so the PE/Act pipeline never
            drains at rowgroup boundaries."""
            r0 = g * RG
            st = {}

            def s_copies():
                st["ot"] = fpool.tile([HD, RG], bf16, name="ot_sb" + r,
                                      tag="ot")
                nc.vector.tensor_copy(st["ot"][:], acc[0:HD, :])
                st["s"] = fpool.tile([1, RG], f32, name="s_sb" + r, tag="s")
                nc.vector.tensor_copy(st["s"][:], acc[HD:HD + 1, :])

            def s_recip():
                stp = eps.tile([128, NJ], f32, name="st_ps" + r, tag="es")
                for j in range(NJ):
                    nc.tensor.transpose(
                        stp[:, j:j + 1],
                        st["s"][0:1, j * 128:(j + 1) * 128],
                        ident11[:])
                st["rt"] = fpool.tile([128, NJ], f32, name="rt_sb" + r,
                                      tag="rt")
                nc.vector.reciprocal(st["rt"][:], stp[:])
                st["out"] = opool.tile([128, NJ, F_OUT], bf16,
                                       name="out_t" + r, tag="out")

            def s_wo(j):
                def f():
                    pso = eps.tile([128, F_OUT], f32, name="pso" + r,
                                   tag="es")
                    nc.tensor.matmul(
                        pso[:],
                        lhsT=st["ot"][:, j * 128:(j + 1) * 128],
                        rhs=wo_sb[:],
                        start=True, stop=True)
                    nc.vector.tensor_scalar_mul(st["out"][:, j, :], pso[:],
                                                st["rt"][:, j:j + 1])
                return f

            def s_store():
                h = NJ // 2
                nc.sync.dma_start(
                    OUT[r0:r0 + RG // 2, :].rearrange(
                        "(j p) f -> p j f", j=h),
                    st["out"][:, 0:h, :])
                nc.gpsimd.dma_start(
                    OUT[r0 + RG // 2:r0 + RG, :].rearrange(
                        "(j p) f -> p j f", j=h),
                    st["out"][:, h:NJ, :])

            return ([s_copies, s_recip]
                    + [s_wo(j) for j in range(NJ)] + [s_store])

        # Flat pipeline over all (g, c) chunks: E/exp/mask lead, PV lags
        # LAG chunks behind (crossing rowgroup boundaries), the previous
        # rowgroup's finalize interleaves one step per slot.
        NTOT = N_RG * N_MC
        accs = {}
        pts = {}
        pending = []
        for k in range(NTOT + LAG):
            if k < NTOT:
                g, c = divmod(k, N_MC)
                r0 = g * RG
                b = c // MB
                if variant != "nomaskdma":
                    # prefetch ~3 batches ahead
                    nxt = (g, b + 3) if b + 3 < NB else (g + 1, b + 3 - NB)
                    if c % MB == 0 and nxt[0] < N_RG and nxt not in mt_tiles:
                        mt_tiles[nxt] = mask_batch(*nxt)
                    if (g, b) not in mt_tiles:
                        mt_tiles[(g, b)] = mask_batch(g, b)
                mt4 = mt_tiles[(g, b)]
                es = eps.tile([128, RG], f32, name="es" + r, tag="es")
                for s in range(RG // NSPLIT):
                    nc.tensor.matmul(
                        es[:, s * NSPLIT:(s + 1) * NSPLIT],
                        lhsT=kt_sb[:, c * 128:(c + 1) * 128],
                        rhs=qt_sb[:, r0 + s * NSPLIT:
                                  r0 + (s + 1) * NSPLIT],
                        start=True, stop=True)
                p_t = ppool.tile([128, RG], bf16, name="p_t" + r, tag="p")
                on_pool = ((c * POOL_EXP) // N_MC
                           != ((c + 1) * POOL_EXP) // N_MC)
                if on_pool:
                    # DVE drains E from PSUM and applies mask additively
                    # (mask*240 + E; exp bias -30 kills masked entries),
                    # then the otherwise-idle gpsimd engine runs the exp.
                    tmp = tpool.tile([128, RG], f32, name="tmp" + r,
                                     tag="tmp")
                    nc.vector.scalar_tensor_tensor(
                        tmp[:], mt4[:, c % MB, :], 30.0 / SCALE, es[:],
                        op0=mybir.AluOpType.mult, op1=mybir.AluOpType.add)
                    _pool_activation(nc, mybir, p_t[:], tmp[:], AF.Exp,
                                     m30_p[:], SCALE)
                else:
                    nc.scalar.activation(p_t[:], es[:], AF.Exp,
                                         bias=zeros_p[:], scale=SCALE)
                    nc.vector.tensor_mul(p_t[:], p_t[:], mt4[:, c % MB, :])
                pts[k] = p_t
            if k >= LAG:
                kv = k - LAG
                gv, cv = divmod(kv, N_MC)
                if cv == 0:
                    accs[gv] = accps.tile([HD + 1, RG], f32, name="acc" + r,
                                          tag="acc")
                p_t = pts.pop(kv)
                for s in range(RG // NSPLIT):
                    nc.tensor.matmul(
                        accs[gv][:, s * NSPLIT:(s + 1) * NSPLIT],
                        lhsT=v_sb[:, cv, 0:HD + 1],
                        rhs=p_t[:, s * NSPLIT:(s + 1) * NSPLIT],
                        start=(cv == 0), stop=(cv == N_MC - 1),
                        skip_group_check=True)
                if cv == N_MC - 1:
                    for f in pending:   # rare leftover (shouldn't happen)
                        f()
                    pending = finalize_steps(gv, accs.pop(gv))
            if pending:
                pending.pop(0)()
        for f in pending:
            f()
        ctx3.__exit__(None, None, None)
        ctx2.__exit__(None, None, None)


def _shard_inputs(X, mask, W_Q, W_K, W_V, W_O):
    """Per-core input dicts (host-side layout prep)."""
    in_maps = []
    for h in range(H):
        xt = X[h].T.astype(BF16)                               # [512, 4096]
        # mask[h].T as bf16 bits: 1 -> 0x3F80 (bf16 1.0), 0 -> 0
        m16 = mask[h].view(np.uint16)[:, 0::2]                 # low half of i32
        mt = (m16.T * np.uint16(0x3F80)).view(BF16)            # [4096, 4096]
        # tile: [keys, queries] -> [g, b, 512, 1024] contiguous per batch
        mt = np.ascontiguousarray(
            mt.reshape(NB, MB * 128, N_RG, RG).transpose(2, 0, 1, 3))
        wq = W_Q[h].T.reshape(4, 128, HD).transpose(1, 0, 2)   # [128, 4, 64]
        wk = W_K[h].T.reshape(4, 128, HD).transpose(1, 0, 2)
        wqk = np.concatenate([wq, wk], axis=2).astype(BF16)    # [128, 4, 128]
        wv = np.ascontiguousarray(
            W_V[h].T.reshape(4, 128, HD).transpose(1, 0, 2)).astype(BF16)
        wo = W_O[:, h * HD:(h + 1) * HD].T.astype(BF16)        # [64, 512]
        in_maps.append({"xt": xt, "mt": mt, "wqk": wqk,
                        "wv": wv, "wo": wo})
    return in_maps


def kernel(X, mask, W_Q, W_K, W_V, W_O):
    from concourse.bass_utils import run_bass_kernel_spmd
    nc = _build_program(repeat=1)
    in_maps = _shard_inputs(X, mask, W_Q, W_K, W_V, W_O)
    res = run_bass_kernel_spmd(nc, in_maps, list(range(N_CORES)))
    out = np.zeros((N, F_OUT), np.float64)
    for h in range(H):
        out += res.results[h]["out"].astype(np.float64)
    return out.astype(np.float32)
